# revision 27
# baseline (speedup 1.0000x reference)
"""CMRGCN Trainium2 kernel, v2.

Sharding: data-parallel over batch B=8 across the 8 NeuronCores (core b gets
batch b). Adjacency / neighbor weights / fused relation weights are replicated.

v2 structure (vs baseline):
  - all matmul operands bf16 (FWL fast weight loads; fp32 PSUM accumulation)
  - layer loop software-pipelined over t-pairs: adjacency matmuls for pair p
    interleave with weight matmuls + activations for pair p-1 and h-adds
  - activations batched over t-pairs ([128, 2, 500] per op)
  - h -> g layout flip via the DMA xbar transpose engine (frees PE + PSUM)
  - neighbor gather runs with densified W_g as the STATIONARY operand and g
    streaming, producing node-major output [m, (t, i, d)]; PSUM evacuation and
    the output DMA are fully contiguous; final layout fixup happens on host
  - PSUM static budget: psA singles bufs=2 (2 banks) + psW pairs bufs=3
    (6 banks) = 8 banks; the gather reuses the psW name
"""

import os
import numpy as np

B, T, N, DIM = 8, 12, 500, 64
N_MIX, N_LAYERS, N_HEADS, N_REL, NG, K = 2, 2, 4, 8, 2, 20
NP = 512          # padded node count
KT = NP // 128    # node tiles
C = DIM * (N_LAYERS + 1)   # 192 channels per mix in g
NCORES = 8
NB = N_MIX * DIM  # 128: (j, d) channel block
TW = T * NB       # 1536: per-cb gather row chunk

_BUILT = {}


def _rel(tg, i, j):
    return (tg * N_MIX + i) * N_MIX + j


def _build():
    """Build + trace the single-core SPMD Bass program once."""
    if "nc" in _BUILT:
        return _BUILT["nc"]

    from contextlib import ExitStack
    import concourse.bass as bass
    import concourse.tile as tile
    import concourse.mybir as mybir
    from concourse import bacc
    from concourse.masks import make_identity

    f32 = mybir.dt.float32
    bf16 = mybir.dt.bfloat16
    AF = mybir.ActivationFunctionType
    ALU = mybir.AluOpType

    nc = bacc.Bacc("TRN2", target_bir_lowering=False, debug=False)

    xn_d = nc.dram_tensor("xn", [NP, T, N_MIX, DIM], bf16, kind="ExternalInput").ap()
    adj_d = nc.dram_tensor("adj", [NG, 128, KT, NP], bf16, kind="ExternalInput").ap()
    wg_d = nc.dram_tensor("wg", [NG, 128, KT, KT, 128], bf16, kind="ExternalInput").ap()
    wmm_d = nc.dram_tensor("wmm", [128, 12, 128], bf16, kind="ExternalInput").ap()
    bias_d = nc.dram_tensor("bias", [128, 16], f32, kind="ExternalInput").ap()
    outg_d = nc.dram_tensor(
        "outg", [NG, 3, KT, 128, TW], bf16, kind="ExternalOutput"
    ).ap()

    with tile.TileContext(nc) as tc, ExitStack() as ctx:
        wpool = ctx.enter_context(tc.tile_pool(name="wpool", bufs=1))
        gpool = ctx.enter_context(tc.tile_pool(name="gpool", bufs=1))
        gmpool = ctx.enter_context(tc.tile_pool(name="gmpool", bufs=1))
        ptpool = ctx.enter_context(tc.tile_pool(name="ptpool", bufs=1))
        hpool = ctx.enter_context(tc.tile_pool(name="hpool", bufs=1))
        tmpool = ctx.enter_context(tc.tile_pool(name="tmpool", bufs=2))
        accpool = ctx.enter_context(tc.tile_pool(name="accpool", bufs=2))
        outpool = ctx.enter_context(tc.tile_pool(name="outpool", bufs=4))
        psA = ctx.enter_context(tc.tile_pool(name="psA", bufs=2, space="PSUM"))
        psW = ctx.enter_context(tc.tile_pool(name="psW", bufs=2, space="PSUM"))
        psT = ctx.enter_context(tc.tile_pool(name="psT", bufs=2, space="PSUM"))

        # --- constants / weights to SBUF ---
        wmm_sb = wpool.tile([128, 12, 128], bf16, name="wmm_sb")
        nc.sync.dma_start(out=wmm_sb[:], in_=wmm_d[:])
        bias_sb = wpool.tile([128, 16], f32, name="bias_sb")
        nc.sync.dma_start(out=bias_sb[:], in_=bias_d[:])
        ident = wpool.tile([128, 128], bf16, name="ident")
        make_identity(nc, ident[:])

        # HAM warmup + startup fill: throwaway matmuls on locally-generated
        # tiles (no DMA dependency at all) keep the PE busy at full clock
        # while the x / adjacency / gather-weight DMAs stream in (~10us)
        zeros = wpool.tile([128, 2, NP], bf16, name="zeros")
        nc.gpsimd.memset(zeros[:], 0.0)
        for w in range(30):
            wa = psA.tile([128, NP], f32, name="psA")
            nc.tensor.matmul(
                wa[:], ident[:], zeros[:, 0, :], start=True, stop=True
            )

        def c_bias(l, tg, j):
            col = (l * NG + tg) * 2 + j
            return bias_sb[:, col:col + 1]

        def d_bias(l, tg):
            return bias_sb[:, 8 + l * NG + tg: 8 + l * NG + tg + 1]

        def hconst(l):
            return bias_sb[:, 12 + l:12 + l + 1]

        # --- g tiles: [m, cb, t, j, d]; x into c-block 0 via one DMA per mt ---
        g = []
        for mt in range(KT):
            gt = gpool.tile([128, 3, T, N_MIX, DIM], bf16, name=f"g{mt}")
            g.append(gt)
        # h pads (cols 500:512) must stay zero: adds only write [:, :, :500]
        h = hpool.tile([128, T, NP], bf16, name="h")
        nc.gpsimd.memset(h[:], 0.0)

        # interleave x / adjacency chunk loads so the first adjacency matmuls
        # can start before the full input set lands
        adj_sb = [
            gmpool.tile([128, KT, NP], bf16, name=f"adj{tg}") for tg in range(NG)
        ]
        for mt in range(KT):
            nc.sync.dma_start(
                out=g[mt][:, 0, :, :, :],
                in_=xn_d[mt * 128:(mt + 1) * 128, :, :, :],
            )
            for tg in range(NG):
                nc.sync.dma_start(
                    out=adj_sb[tg][:, mt, :], in_=adj_d[tg, :, mt, :]
                )
        wg_sb = []
        for tg in range(NG):
            w = gmpool.tile([128, KT, KT, 128], bf16, name=f"wg{tg}")
            nc.sync.dma_start(out=w[:], in_=wg_d[tg])
            wg_sb.append(w)

        ptg = [
            ptpool.tile([128, T, NP], bf16, name=f"ptg{tg}") for tg in range(NG)
        ]

        NPAIR = T // 2  # 6
        deferred_transposes = []

        # ---------------- layers (software-pipelined over t-pairs) ----------
        for l in range(N_LAYERS):
            terms_by_pair = {}

            def a_half(p, s, l=l):
                """Adjacency matmuls + P^T copy for slot s of pair p."""
                t = 2 * p + s
                pa = [psA.tile([128, NP], f32, name="psA") for _ in range(NG)]
                for kt in range(KT):
                    for tg in range(NG):
                        nc.tensor.matmul(
                            pa[tg][:, :500],
                            g[kt][:, l, t, :, :],
                            adj_sb[tg][:, kt, :500],
                            start=(kt == 0),
                            stop=(kt == KT - 1),
                        )
                nc.scalar.copy(ptg[0][:, t, :500], pa[0][:, :500])
                nc.vector.tensor_copy(ptg[1][:, t, :500], pa[1][:, :500])

            def w_blk(p, tg, kind, j, l=l):
                """Weight matmuls + paired activation: one (tg, block)."""
                t0 = 2 * p
                terms = terms_by_pair.setdefault(p, {})
                if kind == "c":
                    blk = (l * NG + tg) * 2 + j
                    fn, bias = AF.Relu, c_bias(l, tg, j)
                    nm = f"r{tg}{j}"
                else:
                    blk = 8 + l * NG + tg
                    fn, bias = AF.Tanh, d_bias(l, tg)
                    nm = f"d{tg}"
                pw = psW.tile([128, 2, NP], f32, name="psW")
                for s in (0, 1):
                    nc.tensor.matmul(
                        pw[:, s, :500],
                        wmm_sb[:, blk, :],
                        ptg[tg][:, t0 + s, :500],
                        start=True,
                        stop=True,
                    )
                tm = tmpool.tile([128, 2, NP], bf16, name=nm)
                if nm == "r11":
                    # offload one relu to the DVE: max(psum + bias, 0)
                    nc.vector.scalar_tensor_tensor(
                        tm[:, :, :500], pw[:, :, :500], bias,
                        zeros[:, :, :500], op0=ALU.add, op1=ALU.max,
                    )
                else:
                    nc.scalar.activation(
                        tm[:, :, :500], pw[:, :, :500], fn, bias=bias
                    )
                terms[nm] = tm

            def adds(p, l=l):
                """h accumulation for pair p (DVE start, GPSIMD finish)."""
                t0 = 2 * p
                terms = terms_by_pair.pop(p)

                def V(nm):
                    return terms[nm][:, :, :500]

                # bf16 accumulator: 2x DVE rate; rounding error is far below
                # the bf16 matmul-input quantization already present
                acc = accpool.tile([128, 2, NP], bf16, name="acc")
                av = acc[:, :, :500]
                nc.vector.scalar_tensor_tensor(
                    av, V("r00"), hconst(l), V("r01"), op0=ALU.add, op1=ALU.add
                )
                nc.vector.tensor_add(av, av, V("r10"))
                nc.vector.tensor_add(av, av, V("r11"))
                nc.vector.tensor_add(av, av, V("d0"))
                nc.vector.tensor_add(h[:, t0:t0 + 2, :500], av, V("d1"))

            def transposes(p, l=l):
                """PE transposes of the pair's h back into g c-block l+1."""
                t0 = 2 * p
                for mt in range(KT):
                    pt = psT.tile([128, 2, 128], bf16, name="psT")
                    for s in (0, 1):
                        nc.tensor.transpose(
                            pt[:, s, :],
                            h[:, t0 + s, mt * 128:(mt + 1) * 128],
                            ident[:],
                        )
                    dst = g[mt][:, l + 1, t0:t0 + 2, :, :]
                    src = pt[:].rearrange("p s (j d) -> p s j d", j=N_MIX)
                    if mt % 2 == 0:
                        nc.scalar.copy(dst, src)
                    else:
                        nc.vector.tensor_copy(dst, src)

            # pipeline: A-matmuls for pair p interleave with W-blocks and
            # h-adds/transposes for pair p-1; W-blocks are spread between the
            # A-halves so the 2-deep psW rotation never stalls the PE.
            # Layer-1 transposes only feed the gather, so they are deferred
            # into the gather phase to shorten the layer critical path.
            BLKS = [(tg, kind, j) for tg in range(NG)
                    for kind, j in (("c", 0), ("c", 1), ("d", None))]
            for p in range(NPAIR + 1):
                if p < NPAIR:
                    a_half(p, 0)
                if p >= 1:
                    for tg, kind, j in BLKS[:3]:
                        w_blk(p - 1, tg, kind, j)
                if p < NPAIR:
                    a_half(p, 1)
                if p >= 1:
                    for tg, kind, j in BLKS[3:]:
                        w_blk(p - 1, tg, kind, j)
                    adds(p - 1)
                    if l == 0:
                        transposes(p - 1)
                    else:
                        deferred_transposes.append((transposes, p - 1))

        # ---------------- neighbor gather (node-major, W_g stationary) ------
        # chunks c0/c1 evacuate from the psA name, c2 from psW, so the 2-deep
        # rotations of both names overlap consecutive units; layer-1 h->g
        # transposes are interspersed here (the gather cb<=1 waves don't need
        # them, the cb=2 wave does).
        unit = 0
        for cb in range(3):
            for tg in range(NG):
                for mb in range(KT):
                    pa0 = psA.tile([128, NP], f32, name="psA")
                    pa1 = psA.tile([128, NP], f32, name="psA")
                    pw = psW.tile([128, 2, NP], f32, name="psW")
                    chunks = [pa0[:, :], pa1[:, :], pw[:, 0, :]]
                    osb = outpool.tile([128, TW], bf16, name="osb")
                    if unit % 2 == 0:
                        cps = [nc.scalar.copy, nc.vector.tensor_copy,
                               nc.scalar.copy]
                    else:
                        cps = [nc.vector.tensor_copy, nc.scalar.copy,
                               nc.vector.tensor_copy]
                    for kt in range(KT):
                        for c in range(3):
                            nc.tensor.matmul(
                                chunks[c],
                                wg_sb[tg][:, kt, mb, :],
                                g[kt][:, cb, 4 * c:4 * c + 4, :, :],
                                start=(kt == 0),
                                stop=(kt == KT - 1),
                            )
                            if kt == KT - 1:
                                # evacuate each chunk as soon as it completes
                                cps[c](osb[:, c * NP:(c + 1) * NP], chunks[c])
                                # per-chunk DMA: the phase tail only waits on
                                # the final chunk, not the whole unit
                                nc.sync.dma_start(
                                    out=outg_d[tg, cb, mb, :,
                                               c * NP:(c + 1) * NP],
                                    in_=osb[:, c * NP:(c + 1) * NP],
                                )
                    if deferred_transposes:
                        fn, p = deferred_transposes.pop(0)
                        fn(p)
                    unit += 1

    nc.compile()
    _BUILT["nc"] = nc
    return nc


def _host_prep(x0, x1, graphs, neighbors, neighbors_weight, a_weight, B_weight,
               a_bias, B_bias):
    """Fuse weights, densify gather, build per-core input maps."""
    import ml_dtypes
    bf16 = ml_dtypes.bfloat16
    f = np.float32
    x0 = np.asarray(x0, f)
    x1 = np.asarray(x1, f)
    graphs = np.asarray(graphs, f)
    neighbors = np.asarray(neighbors).astype(np.int64)
    neighbors_weight = np.asarray(neighbors_weight, f)
    a_weight = np.asarray(a_weight, f)
    B_weight = np.asarray(B_weight, f)
    a_bias = np.asarray(a_bias, f)
    B_bias = np.asarray(B_bias, f)

    # fused relation weights: wc/wd [R, L, D, D], bc/bd [R, L, D]
    wc = np.sum(a_weight[0] * B_weight, axis=1)
    wd = np.sum(a_weight[1] * B_weight, axis=1)
    bc = np.sum(a_bias[0] * B_bias, axis=1)
    bd = np.sum(a_bias[1] * B_bias, axis=1)
    # wmm blob: 12 blocks of [128=(j,d), 128=(i,d')], K=128 with zero halves.
    wmm = np.zeros((128, 12, 128), f)
    for l in range(N_LAYERS):
        for tg in range(NG):
            for j in range(N_MIX):
                blk = (l * NG + tg) * 2 + j
                r0 = j * 64
                wmm[r0:r0 + 64, blk, 0:64] = wc[_rel(tg, 0, j), l]
                wmm[r0:r0 + 64, blk, 64:128] = wc[_rel(tg, 1, j), l]
            blk = 8 + l * NG + tg
            wd01, wd10 = wd[_rel(tg, 0, 1), l], wd[_rel(tg, 1, 0), l]
            wmm[0:64, blk, 0:64] = -wd01
            wmm[0:64, blk, 64:128] = wd10
            wmm[64:128, blk, 0:64] = wd01
            wmm[64:128, blk, 64:128] = -wd10

    bias = np.zeros((128, 16), f)
    for l in range(N_LAYERS):
        for tg in range(NG):
            for j in range(N_MIX):
                col = (l * NG + tg) * 2 + j
                bias[0:64, col] = bc[_rel(tg, 0, j), l]
                bias[64:128, col] = bc[_rel(tg, 1, j), l]
            col = 8 + l * NG + tg
            bias[0:64, col] = bd[_rel(tg, 0, 1), l]
            bias[64:128, col] = bd[_rel(tg, 1, 0), l]
        hc = np.zeros(128, f)
        for i in range(N_MIX):
            acc = np.zeros(DIM, f)
            for tg in range(NG):
                acc += np.tanh(bd[_rel(tg, i, i), l])
            hc[i * DIM:(i + 1) * DIM] = acc
        bias[:, 12 + l] = hc

    # adjacency: [tg, p, kt, m] with n = kt*128 + p
    adjp = np.zeros((NG, NP, NP), f)
    adjp[:, :N, :N] = graphs
    adj_in = np.ascontiguousarray(
        adjp.reshape(NG, KT, 128, NP).transpose(0, 2, 1, 3)
    ).astype(bf16)

    # densified gather weights: wgp[tg][n, m] = sum of neighbor weights
    wgp = np.zeros((NG, NP, NP), f)
    for tg in range(NG):
        np.add.at(
            wgp[tg],
            (neighbors[tg].reshape(-1), np.repeat(np.arange(N), K)),
            neighbors_weight[tg].reshape(-1),
        )
    # [tg, p, kt, mb, c] with n = kt*128 + p, m = mb*128 + c
    wg_in = np.ascontiguousarray(
        wgp.reshape(NG, KT, 128, KT, 128).transpose(0, 2, 1, 3, 4)
    ).astype(bf16)

    wmm_in = wmm.astype(bf16)

    in_maps = []
    for b in range(NCORES):
        xn = np.zeros((NP, T, N_MIX, DIM), f)
        # [j, D, N, T] -> [N, T, j, D]
        xn[:N] = np.stack([x0[b], x1[b]], axis=0).transpose(2, 3, 0, 1)
        in_maps.append({
            "xn": xn.astype(bf16), "adj": adj_in, "wg": wg_in,
            "wmm": wmm_in, "bias": bias,
        })
    return in_maps


def kernel(x0, x1, graphs, neighbors, neighbors_weight, a_weight, B_weight,
           a_bias, B_bias):
    from concourse.bass_utils import run_bass_kernel_spmd

    nc = _build()
    in_maps = _host_prep(x0, x1, graphs, neighbors, neighbors_weight,
                         a_weight, B_weight, a_bias, B_bias)
    trace = bool(int(os.environ.get("KERNEL_TRACE", "0")))
    res = run_bass_kernel_spmd(nc, in_maps, list(range(NCORES)), trace=trace)
    kernel.last_result = res

    outs = []
    for b in range(NCORES):
        o = np.asarray(res.results[b]["outg"]).astype(np.float32)
        # [tg, cb, mb, p, t, j, d] -> [j, (tg cb d), (mb p), t]
        o = o.reshape(NG, 3, KT, 128, T, N_MIX, DIM)
        o = o.transpose(5, 0, 1, 6, 2, 3, 4).reshape(N_MIX, NG * C, NP, T)
        outs.append(o[:, :, :N, :])
    out = np.stack(outs, axis=1)  # [mix, B, C*NG, N, T]
    return out[0], out[1]


kernel.last_result = None


# revision 29
# speedup vs baseline: 1.0039x; 1.0039x over previous
"""CMRGCN Trainium2 kernel, v2.

Sharding: data-parallel over batch B=8 across the 8 NeuronCores (core b gets
batch b). Adjacency / neighbor weights / fused relation weights are replicated.

v2 structure (vs baseline):
  - all matmul operands bf16 (FWL fast weight loads; fp32 PSUM accumulation)
  - layer loop software-pipelined over t-pairs: adjacency matmuls for pair p
    interleave with weight matmuls + activations for pair p-1 and h-adds
  - activations batched over t-pairs ([128, 2, 500] per op)
  - h -> g layout flip via the DMA xbar transpose engine (frees PE + PSUM)
  - neighbor gather runs with densified W_g as the STATIONARY operand and g
    streaming, producing node-major output [m, (t, i, d)]; PSUM evacuation and
    the output DMA are fully contiguous; final layout fixup happens on host
  - PSUM static budget: psA singles bufs=2 (2 banks) + psW pairs bufs=3
    (6 banks) = 8 banks; the gather reuses the psW name
"""

import os
import numpy as np

B, T, N, DIM = 8, 12, 500, 64
N_MIX, N_LAYERS, N_HEADS, N_REL, NG, K = 2, 2, 4, 8, 2, 20
NP = 512          # padded node count
KT = NP // 128    # node tiles
C = DIM * (N_LAYERS + 1)   # 192 channels per mix in g
NCORES = 8
NB = N_MIX * DIM  # 128: (j, d) channel block
TW = T * NB       # 1536: per-cb gather row chunk

_BUILT = {}


def _rel(tg, i, j):
    return (tg * N_MIX + i) * N_MIX + j


def _build():
    """Build + trace the single-core SPMD Bass program once."""
    if "nc" in _BUILT:
        return _BUILT["nc"]

    from contextlib import ExitStack
    import concourse.bass as bass
    import concourse.tile as tile
    import concourse.mybir as mybir
    from concourse import bacc
    from concourse.masks import make_identity

    f32 = mybir.dt.float32
    bf16 = mybir.dt.bfloat16
    AF = mybir.ActivationFunctionType
    ALU = mybir.AluOpType

    nc = bacc.Bacc("TRN2", target_bir_lowering=False, debug=False)

    xn_d = nc.dram_tensor("xn", [NP, T, N_MIX, DIM], bf16, kind="ExternalInput").ap()
    adj_d = nc.dram_tensor("adj", [NG, 128, KT, NP], bf16, kind="ExternalInput").ap()
    wg_d = nc.dram_tensor("wg", [NG, 128, KT, KT, 128], bf16, kind="ExternalInput").ap()
    wmm_d = nc.dram_tensor("wmm", [128, 12, 128], bf16, kind="ExternalInput").ap()
    bias_d = nc.dram_tensor("bias", [128, 16], f32, kind="ExternalInput").ap()
    outg_d = nc.dram_tensor(
        "outg", [NG, 3, KT, 128, TW], bf16, kind="ExternalOutput"
    ).ap()

    with tile.TileContext(nc) as tc, ExitStack() as ctx:
        wpool = ctx.enter_context(tc.tile_pool(name="wpool", bufs=1))
        gpool = ctx.enter_context(tc.tile_pool(name="gpool", bufs=1))
        gmpool = ctx.enter_context(tc.tile_pool(name="gmpool", bufs=1))
        ptpool = ctx.enter_context(tc.tile_pool(name="ptpool", bufs=1))
        hpool = ctx.enter_context(tc.tile_pool(name="hpool", bufs=1))
        tmpool = ctx.enter_context(tc.tile_pool(name="tmpool", bufs=2))
        accpool = ctx.enter_context(tc.tile_pool(name="accpool", bufs=2))
        outpool = ctx.enter_context(tc.tile_pool(name="outpool", bufs=4))
        psA = ctx.enter_context(tc.tile_pool(name="psA", bufs=2, space="PSUM"))
        psW = ctx.enter_context(tc.tile_pool(name="psW", bufs=2, space="PSUM"))
        psT = ctx.enter_context(tc.tile_pool(name="psT", bufs=2, space="PSUM"))

        # --- constants / weights to SBUF ---
        wmm_sb = wpool.tile([128, 12, 128], bf16, name="wmm_sb")
        nc.sync.dma_start(out=wmm_sb[:], in_=wmm_d[:])
        bias_sb = wpool.tile([128, 16], f32, name="bias_sb")
        nc.sync.dma_start(out=bias_sb[:], in_=bias_d[:])
        ident = wpool.tile([128, 128], bf16, name="ident")
        make_identity(nc, ident[:])

        # HAM warmup + startup fill: throwaway matmuls on locally-generated
        # tiles (no DMA dependency at all) keep the PE busy at full clock
        # while the x / adjacency / gather-weight DMAs stream in (~10us)
        zeros = wpool.tile([128, 2, NP], bf16, name="zeros")
        nc.gpsimd.memset(zeros[:], 0.0)
        for w in range(30):
            wa = psA.tile([128, NP], f32, name="psA")
            nc.tensor.matmul(
                wa[:], ident[:], zeros[:, 0, :], start=True, stop=True
            )

        def c_bias(l, tg, j):
            col = (l * NG + tg) * 2 + j
            return bias_sb[:, col:col + 1]

        def d_bias(l, tg):
            return bias_sb[:, 8 + l * NG + tg: 8 + l * NG + tg + 1]

        def hconst(l):
            return bias_sb[:, 12 + l:12 + l + 1]

        # --- g tiles: [m, cb, t, j, d]; x into c-block 0 via one DMA per mt ---
        g = []
        for mt in range(KT):
            gt = gpool.tile([128, 3, T, N_MIX, DIM], bf16, name=f"g{mt}")
            g.append(gt)
        # h pads (cols 500:512) must stay zero: adds only write [:, :, :500]
        h = hpool.tile([128, T, NP], bf16, name="h")
        nc.gpsimd.memset(h[:], 0.0)

        # interleave x / adjacency chunk loads so the first adjacency matmuls
        # can start before the full input set lands
        adj_sb = [
            gmpool.tile([128, KT, NP], bf16, name=f"adj{tg}") for tg in range(NG)
        ]
        for mt in range(KT):
            nc.sync.dma_start(
                out=g[mt][:, 0, :, :, :],
                in_=xn_d[mt * 128:(mt + 1) * 128, :, :, :],
            )
            for tg in range(NG):
                nc.sync.dma_start(
                    out=adj_sb[tg][:, mt, :], in_=adj_d[tg, :, mt, :]
                )
        wg_sb = []
        for tg in range(NG):
            w = gmpool.tile([128, KT, KT, 128], bf16, name=f"wg{tg}")
            nc.sync.dma_start(out=w[:], in_=wg_d[tg])
            wg_sb.append(w)

        ptg = [
            ptpool.tile([128, T, NP], bf16, name=f"ptg{tg}") for tg in range(NG)
        ]

        NPAIR = T // 2  # 6
        deferred_transposes = []

        # ---------------- layers (software-pipelined over t-pairs) ----------
        for l in range(N_LAYERS):
            terms_by_pair = {}

            def a_half(p, s, l=l):
                """Adjacency matmuls + P^T copy for slot s of pair p."""
                t = 2 * p + s
                pa = [psA.tile([128, NP], f32, name="psA") for _ in range(NG)]
                for kt in range(KT):
                    for tg in range(NG):
                        nc.tensor.matmul(
                            pa[tg][:, :500],
                            g[kt][:, l, t, :, :],
                            adj_sb[tg][:, kt, :500],
                            start=(kt == 0),
                            stop=(kt == KT - 1),
                        )
                nc.scalar.copy(ptg[0][:, t, :500], pa[0][:, :500])
                nc.vector.tensor_copy(ptg[1][:, t, :500], pa[1][:, :500])

            def w_blk(p, tg, kind, j, l=l):
                """Weight matmuls + paired activation: one (tg, block)."""
                t0 = 2 * p
                terms = terms_by_pair.setdefault(p, {})
                if kind == "c":
                    blk = (l * NG + tg) * 2 + j
                    fn, bias = AF.Relu, c_bias(l, tg, j)
                    nm = f"r{tg}{j}"
                else:
                    blk = 8 + l * NG + tg
                    fn, bias = AF.Tanh, d_bias(l, tg)
                    nm = f"d{tg}"
                pw = psW.tile([128, 2, NP], f32, name="psW")
                for s in (0, 1):
                    nc.tensor.matmul(
                        pw[:, s, :500],
                        wmm_sb[:, blk, :],
                        ptg[tg][:, t0 + s, :500],
                        start=True,
                        stop=True,
                    )
                tm = tmpool.tile([128, 2, NP], bf16, name=nm)
                if nm == "r11":
                    # offload one relu to the DVE: max(psum + bias, 0)
                    nc.vector.scalar_tensor_tensor(
                        tm[:, :, :500], pw[:, :, :500], bias,
                        zeros[:, :, :500], op0=ALU.add, op1=ALU.max,
                    )
                else:
                    nc.scalar.activation(
                        tm[:, :, :500], pw[:, :, :500], fn, bias=bias
                    )
                terms[nm] = tm

            def adds(p, l=l):
                """h accumulation for pair p (DVE start, GPSIMD finish)."""
                t0 = 2 * p
                terms = terms_by_pair.pop(p)

                def V(nm):
                    return terms[nm][:, :, :500]

                # bf16 accumulator: 2x DVE rate; rounding error is far below
                # the bf16 matmul-input quantization already present
                acc = accpool.tile([128, 2, NP], bf16, name="acc")
                av = acc[:, :, :500]
                nc.vector.scalar_tensor_tensor(
                    av, V("r00"), hconst(l), V("r01"), op0=ALU.add, op1=ALU.add
                )
                nc.vector.tensor_add(av, av, V("r10"))
                nc.vector.tensor_add(av, av, V("r11"))
                nc.vector.tensor_add(av, av, V("d0"))
                nc.vector.tensor_add(h[:, t0:t0 + 2, :500], av, V("d1"))

            def transposes(p, l=l):
                """PE transposes of the pair's h back into g c-block l+1."""
                t0 = 2 * p
                for mt in range(KT):
                    pt = psT.tile([128, 2, 128], bf16, name="psT")
                    for s in (0, 1):
                        nc.tensor.transpose(
                            pt[:, s, :],
                            h[:, t0 + s, mt * 128:(mt + 1) * 128],
                            ident[:],
                        )
                    dst = g[mt][:, l + 1, t0:t0 + 2, :, :]
                    src = pt[:].rearrange("p s (j d) -> p s j d", j=N_MIX)
                    if mt % 2 == 0:
                        nc.scalar.copy(dst, src)
                    else:
                        nc.vector.tensor_copy(dst, src)

            # pipeline: A-matmuls for pair p interleave with W-blocks and
            # h-adds/transposes for pair p-1; W-blocks are spread between the
            # A-halves so the 2-deep psW rotation never stalls the PE.
            # Layer-1 transposes only feed the gather, so they are deferred
            # into the gather phase to shorten the layer critical path.
            BLKS = [(tg, kind, j) for tg in range(NG)
                    for kind, j in (("c", 0), ("c", 1), ("d", None))]
            for p in range(NPAIR + 1):
                if p < NPAIR:
                    a_half(p, 0)
                if p >= 1:
                    for tg, kind, j in BLKS[:3]:
                        w_blk(p - 1, tg, kind, j)
                if p < NPAIR:
                    a_half(p, 1)
                if p >= 1:
                    for tg, kind, j in BLKS[3:]:
                        w_blk(p - 1, tg, kind, j)
                    adds(p - 1)
                    if l == 0:
                        transposes(p - 1)
                    else:
                        deferred_transposes.append((transposes, p - 1))

        # ---------------- neighbor gather (node-major, W_g stationary) ------
        # chunks c0/c1 evacuate from the psA name, c2 from psW, so the 2-deep
        # rotations of both names overlap consecutive units; layer-1 h->g
        # transposes are interspersed here (the gather cb<=1 waves don't need
        # them, the cb=2 wave does).
        unit = 0
        for cb in range(3):
            for tg in range(NG):
                for mb in range(KT):
                    pa0 = psA.tile([128, NP], f32, name="psA")
                    pa1 = psA.tile([128, NP], f32, name="psA")
                    pw = psW.tile([128, 2, NP], f32, name="psW")
                    chunks = [pa0[:, :], pa1[:, :], pw[:, 0, :]]
                    osb = outpool.tile([128, TW], bf16, name="osb")
                    if unit % 2 == 0:
                        cps = [nc.scalar.copy, nc.vector.tensor_copy,
                               nc.scalar.copy]
                    else:
                        cps = [nc.vector.tensor_copy, nc.scalar.copy,
                               nc.vector.tensor_copy]
                    for kt in range(KT):
                        for c in range(3):
                            nc.tensor.matmul(
                                chunks[c],
                                wg_sb[tg][:, kt, mb, :],
                                g[kt][:, cb, 4 * c:4 * c + 4, :, :],
                                start=(kt == 0),
                                stop=(kt == KT - 1),
                            )
                            if kt == KT - 1:
                                # evacuate each chunk as soon as it completes
                                cps[c](osb[:, c * NP:(c + 1) * NP], chunks[c])
                                # per-chunk DMA: the phase tail only waits on
                                # the final chunk, not the whole unit
                                nc.sync.dma_start(
                                    out=outg_d[tg, cb, mb, :,
                                               c * NP:(c + 1) * NP],
                                    in_=osb[:, c * NP:(c + 1) * NP],
                                )
                    if deferred_transposes:
                        fn, p = deferred_transposes.pop(0)
                        fn(p)
                    unit += 1

    nc.compile()
    _BUILT["nc"] = nc
    return nc


def _host_prep(x0, x1, graphs, neighbors, neighbors_weight, a_weight, B_weight,
               a_bias, B_bias):
    """Fuse weights, densify gather, build per-core input maps."""
    import ml_dtypes
    bf16 = ml_dtypes.bfloat16
    f = np.float32
    x0 = np.asarray(x0, f)
    x1 = np.asarray(x1, f)
    graphs = np.asarray(graphs, f)
    neighbors = np.asarray(neighbors).astype(np.int64)
    neighbors_weight = np.asarray(neighbors_weight, f)
    a_weight = np.asarray(a_weight, f)
    B_weight = np.asarray(B_weight, f)
    a_bias = np.asarray(a_bias, f)
    B_bias = np.asarray(B_bias, f)

    # fused relation weights: wc/wd [R, L, D, D], bc/bd [R, L, D]
    wc = np.sum(a_weight[0] * B_weight, axis=1)
    wd = np.sum(a_weight[1] * B_weight, axis=1)
    bc = np.sum(a_bias[0] * B_bias, axis=1)
    bd = np.sum(a_bias[1] * B_bias, axis=1)
    # wmm blob: 12 blocks of [128=(j,d), 128=(i,d')], K=128 with zero halves.
    wmm = np.zeros((128, 12, 128), f)
    for l in range(N_LAYERS):
        for tg in range(NG):
            for j in range(N_MIX):
                blk = (l * NG + tg) * 2 + j
                r0 = j * 64
                wmm[r0:r0 + 64, blk, 0:64] = wc[_rel(tg, 0, j), l]
                wmm[r0:r0 + 64, blk, 64:128] = wc[_rel(tg, 1, j), l]
            blk = 8 + l * NG + tg
            wd01, wd10 = wd[_rel(tg, 0, 1), l], wd[_rel(tg, 1, 0), l]
            wmm[0:64, blk, 0:64] = -wd01
            wmm[0:64, blk, 64:128] = wd10
            wmm[64:128, blk, 0:64] = wd01
            wmm[64:128, blk, 64:128] = -wd10

    bias = np.zeros((128, 16), f)
    for l in range(N_LAYERS):
        for tg in range(NG):
            for j in range(N_MIX):
                col = (l * NG + tg) * 2 + j
                bias[0:64, col] = bc[_rel(tg, 0, j), l]
                bias[64:128, col] = bc[_rel(tg, 1, j), l]
            col = 8 + l * NG + tg
            bias[0:64, col] = bd[_rel(tg, 0, 1), l]
            bias[64:128, col] = bd[_rel(tg, 1, 0), l]
        hc = np.zeros(128, f)
        for i in range(N_MIX):
            acc = np.zeros(DIM, f)
            for tg in range(NG):
                acc += np.tanh(bd[_rel(tg, i, i), l])
            hc[i * DIM:(i + 1) * DIM] = acc
        bias[:, 12 + l] = hc

    # adjacency: [tg, p, kt, m] with n = kt*128 + p
    adjp = np.zeros((NG, NP, NP), f)
    adjp[:, :N, :N] = graphs
    adj_in = np.ascontiguousarray(
        adjp.reshape(NG, KT, 128, NP).transpose(0, 2, 1, 3)
    ).astype(bf16)

    # densified gather weights: wgp[tg][n, m] = sum of neighbor weights
    wgp = np.zeros((NG, NP, NP), f)
    for tg in range(NG):
        np.add.at(
            wgp[tg],
            (neighbors[tg].reshape(-1), np.repeat(np.arange(N), K)),
            neighbors_weight[tg].reshape(-1),
        )
    # [tg, p, kt, mb, c] with n = kt*128 + p, m = mb*128 + c
    wg_in = np.ascontiguousarray(
        wgp.reshape(NG, KT, 128, KT, 128).transpose(0, 2, 1, 3, 4)
    ).astype(bf16)

    wmm_in = wmm.astype(bf16)

    in_maps = []
    for b in range(NCORES):
        xn = np.zeros((NP, T, N_MIX, DIM), f)
        # [j, D, N, T] -> [N, T, j, D]
        xn[:N] = np.stack([x0[b], x1[b]], axis=0).transpose(2, 3, 0, 1)
        in_maps.append({
            "xn": xn.astype(bf16), "adj": adj_in, "wg": wg_in,
            "wmm": wmm_in, "bias": bias,
        })
    return in_maps


def kernel(x0, x1, graphs, neighbors, neighbors_weight, a_weight, B_weight,
           a_bias, B_bias):
    from concourse.bass_utils import run_bass_kernel_spmd

    nc = _build()
    in_maps = _host_prep(x0, x1, graphs, neighbors, neighbors_weight,
                         a_weight, B_weight, a_bias, B_bias)
    trace = bool(int(os.environ.get("KERNEL_TRACE", "0")))
    res = run_bass_kernel_spmd(nc, in_maps, list(range(NCORES)), trace=trace)
    kernel.last_result = res

    outs = []
    for b in range(NCORES):
        o = np.asarray(res.results[b]["outg"]).astype(np.float32)
        # [tg, cb, mb, p, t, j, d] -> [j, (tg cb d), (mb p), t]
        o = o.reshape(NG, 3, KT, 128, T, N_MIX, DIM)
        o = o.transpose(5, 0, 1, 6, 2, 3, 4).reshape(N_MIX, NG * C, NP, T)
        outs.append(o[:, :, :N, :])
    out = np.stack(outs, axis=1)  # [mix, B, C*NG, N, T]
    return out[0], out[1]


kernel.last_result = None
